# revision 1
# baseline (speedup 1.0000x reference)
"""Multi-head attention block (QKV proj + softmax attention + out proj +
residual + LayerNorm) on 8 Trainium2 NeuronCores.

Sharding:
  Phase A: head-parallel — core c computes heads (2c, 2c+1) for both batch
           elements: Q/K/V projections, scores (transposed layout), exp,
           unnormalized P@V and softmax denominators.
  Phase B: row-parallel — core c computes 512 rows of the flattened (B*L)
           output: per-head normalization, output projection (contracting
           over all 16 heads), residual add and LayerNorm.

Matmul operands in fp16 (PE streams 2-byte operands at full rate); all
accumulation in fp32 PSUM; softmax/LayerNorm arithmetic in fp32.
exp uses a fixed -2.0 bias to keep fp16 P values in range; it cancels in
the softmax normalization since denominators use the same biased values.
"""

import sys

if "/opt/trn_rl_repo" not in sys.path:
    sys.path.insert(0, "/opt/trn_rl_repo")

import ml_dtypes
import numpy as np

import concourse.bass as bass
import concourse.tile as tile
from concourse import bacc, mybir
from concourse.bass_utils import run_bass_kernel_spmd
from concourse.masks import make_identity

B, L, D, H, DQ = 2, 2048, 1024, 16, 64
N_CORES = 8
LN_EPS = 1e-5
F32 = mybir.dt.float32
FP16 = mybir.dt.float16
AF = mybir.ActivationFunctionType
OP = mybir.AluOpType
FP16_NP = np.float16

_cache = {}


def _build_phase_a():
    nc = bacc.Bacc("TRN2", target_bir_lowering=False, debug=False, num_devices=N_CORES)
    xt_d = nc.dram_tensor("xt", [B, D, L], FP16, kind="ExternalInput").ap()
    wq_d = nc.dram_tensor("wq", [D, 128], FP16, kind="ExternalInput").ap()
    wk_d = nc.dram_tensor("wk", [D, 128], FP16, kind="ExternalInput").ap()
    wv_d = nc.dram_tensor("wv", [D, 128], FP16, kind="ExternalInput").ap()
    # rows 0..127: A^T (2 heads x 64), rows 128..129: softmax denominators
    at_d = nc.dram_tensor("at", [B, 130, L], F32, kind="ExternalOutput").ap()

    with tile.TileContext(nc) as tc:
        with tc.tile_pool(name="singles", bufs=1) as singles, \
             tc.tile_pool(name="xt_pool", bufs=2) as xt_pool, \
             tc.tile_pool(name="proj_sb", bufs=2) as proj_sb, \
             tc.tile_pool(name="pt_sb", bufs=6) as pt_sb, \
             tc.tile_pool(name="out_sb", bufs=3) as out_sb, \
             tc.tile_pool(name="ps_mm", bufs=3, space="PSUM") as ps_mm, \
             tc.tile_pool(name="ps_apv", bufs=2, space="PSUM") as ps_apv:
            w_sb = {}
            for nm, d in (("wq", wq_d), ("wk", wk_d), ("wv", wv_d)):
                t = singles.tile([128, 8, 128], FP16, tag=nm)
                nc.sync.dma_start(out=t, in_=d.rearrange("(mc p) h -> p mc h", p=128))
                w_sb[nm] = t
            ident_f = singles.tile([128, 128], F32, tag="ident_f")
            make_identity(nc, ident_f)
            ident = singles.tile([128, 128], FP16, tag="ident")
            nc.vector.tensor_copy(out=ident, in_=ident_f)
            exp_bias = singles.tile([128, 1], F32, tag="exp_bias")
            nc.vector.memset(exp_bias, -8.0)

            xt, qt, kt, vt, vsb = {}, {}, {}, {}, {}
            for b in range(B):
                xt[b] = xt_pool.tile([128, 8, L], FP16, tag="xt", name=f"xt{b}")
                for mc in range(8):
                    nc.sync.dma_start(
                        out=xt[b][:, mc, :],
                        in_=xt_d[b].rearrange("(mc p) l -> p mc l", p=128)[:, mc, :],
                    )
                qt[b] = proj_sb.tile([128, L], FP16, tag="qt", name=f"qt{b}")
                kt[b] = proj_sb.tile([128, L], FP16, tag="kt", name=f"kt{b}")
                vt[b] = proj_sb.tile([128, L], FP16, tag="vt", name=f"vt{b}")
                vsb[b] = proj_sb.tile([128, 2, 16, 128], FP16, tag="vsb", name=f"vsb{b}")

            def proj_group(b, dst, w, it):
                sl = slice(it * 512, (it + 1) * 512)
                ps = ps_mm.tile([128, 512], F32, tag="mm", name="ps")
                for mc in range(8):
                    nc.tensor.matmul(
                        ps, w[:, mc, :], xt[b][:, mc, sl],
                        start=(mc == 0), stop=(mc == 7),
                    )
                nc.vector.tensor_copy(out=dst[:, sl], in_=ps)

            def v_init(b):
                nc.vector.memset(vsb[b], 0.0)
                nc.vector.tensor_copy(
                    out=vsb[b][:, :, :, 64:65], in_=ones_col[:, :, :, :]
                )

            def v_transpose(b, h, jc):
                ps = ps_mm.tile([128, 512], FP16, tag="mm", name="ps")
                nc.tensor.transpose(
                    ps[:, 0:64],
                    vt[b][h * 64:(h + 1) * 64, jc * 128:(jc + 1) * 128],
                    ident[h * 64:(h + 1) * 64, h * 64:(h + 1) * 64],
                )
                nc.vector.tensor_copy(out=vsb[b][:, h, jc, 0:64], in_=ps[:, 0:64])

            ones_col = singles.tile([128, 2, 16, 1], FP16, tag="ones")
            nc.vector.memset(ones_col, 1.0)

            # b=0 projections: Q,K first so the softmax exp stream starts
            # as early as possible, then V (+transposes to natural layout)
            # b=0: K fully (scores need all of KT), Q for the first
            # i-block, then V + transposes (PV needs them); Q(it1-3) and
            # all of b=1 are deferred into the attention loop as fillers
            for it in range(4):
                proj_group(0, kt[0], w_sb["wk"], it)
            proj_group(0, qt[0], w_sb["wq"], 0)
            for it in range(4):
                proj_group(0, vt[0], w_sb["wv"], it)
            v_init(0)
            for jc in range(16):
                for h in range(2):
                    v_transpose(0, h, jc)

            # deferred projection work, interleaved into b=0's attention
            # loop to fill PE stalls while ACT (exp) paces it
            filler = []
            for it in range(1, 4):
                filler.append(lambda it=it: proj_group(0, qt[0], w_sb["wq"], it))
            for it in range(4):
                filler.append(lambda it=it: proj_group(1, qt[1], w_sb["wq"], it))
            for it in range(4):
                filler.append(lambda it=it: proj_group(1, kt[1], w_sb["wk"], it))
            for it in range(4):
                filler.append(lambda it=it: proj_group(1, vt[1], w_sb["wv"], it))
            filler.append(lambda: v_init(1))
            for jc in range(16):
                filler.append(lambda jc=jc: (v_transpose(1, 0, jc), v_transpose(1, 1, jc)))

            nonlocal_pace = [0.0]

            def attention(b, emit_filler):
                for it in range(4):
                    i_sl = slice(it * 512, (it + 1) * 512)
                    apv = [
                        ps_apv.tile([128, 512], F32, tag="apv", name=f"apv{_h}")
                        for _h in range(2)
                    ]
                    def score_pair(jc):
                        # one ST tile holds both heads' scores for this jc —
                        # the shared exp forces the two K=64 matmuls adjacent
                        # so they run concurrently in disjoint PE row groups
                        st = ps_mm.tile([128, 1024], F32, tag="mm", name="st")
                        for h in range(2):
                            hs = slice(h * 64, (h + 1) * 64)
                            nc.tensor.matmul(
                                st[:, h * 512:(h + 1) * 512],
                                kt[b][hs, jc * 128:(jc + 1) * 128],
                                qt[b][hs, i_sl],
                                start=True, stop=True,
                            )
                        return st

                    def pv_pair(jc, ptt):
                        for h in range(2):
                            nc.tensor.matmul(
                                apv[h],
                                vsb[b][:, h, jc, :],
                                ptt[:, h * 512:(h + 1) * 512],
                                start=(jc == 0),
                                stop=(jc == 15),
                            )

                    # PV runs one jc behind exp so its weight load never
                    # waits on an in-flight exp
                    sts = {0: score_pair(0)}
                    if 1 < 16:
                        sts[1] = score_pair(1)
                    ptts = {}
                    for jc in range(16):
                        ptts[jc] = pt_sb.tile([128, 1024], FP16, tag="pt", name="pt")
                        nc.scalar.activation(
                            out=ptts[jc], in_=sts.pop(jc), func=AF.Exp,
                            scale=1.0 / (DQ ** 0.5), bias=exp_bias,
                        )
                        if jc + 2 < 16:
                            sts[jc + 2] = score_pair(jc + 2)
                        if jc >= 1:
                            pv_pair(jc - 1, ptts.pop(jc - 1))
                        if emit_filler:
                            nonlocal_pace[0] += 35.0 / 64.0
                            while filler and nonlocal_pace[0] >= 1.0:
                                nonlocal_pace[0] -= 1.0
                                filler.pop(0)()
                    pv_pair(15, ptts.pop(15))
                    for h in range(2):
                        o_sb = out_sb.tile([65, 512], F32, tag="o")
                        nc.vector.tensor_copy(out=o_sb, in_=apv[h][0:65, :])
                        nc.sync.dma_start(
                            out=at_d[b, h * 64:(h + 1) * 64, i_sl], in_=o_sb[0:64, :]
                        )
                        nc.sync.dma_start(
                            out=at_d[b, 128 + h:129 + h, i_sl], in_=o_sb[64:65, :]
                        )

            attention(0, True)
            while filler:
                filler.pop(0)()
            attention(1, False)
    nc.compile()
    return nc


def _build_phase_b():
    nc = bacc.Bacc("TRN2", target_bir_lowering=False, debug=False, num_devices=N_CORES)
    ROWS = B * L // N_CORES  # 512
    atq_d = nc.dram_tensor("atq", [H * DQ, ROWS], FP16, kind="ExternalInput").ap()
    rdn_d = nc.dram_tensor("rdn", [H * DQ, ROWS], FP16, kind="ExternalInput").ap()
    xr_d = nc.dram_tensor("xr", [ROWS, D], F32, kind="ExternalInput").ap()
    wo_d = nc.dram_tensor("wo", [H * DQ, D], FP16, kind="ExternalInput").ap()
    g_d = nc.dram_tensor("gamma", [D], F32, kind="ExternalInput").ap()
    bt_d = nc.dram_tensor("beta", [D], F32, kind="ExternalInput").ap()
    y_d = nc.dram_tensor("y", [ROWS, D], F32, kind="ExternalOutput").ap()

    with tile.TileContext(nc) as tc:
        with tc.tile_pool(name="sb", bufs=1) as sb, \
             tc.tile_pool(name="yt_sb", bufs=3) as yt_sb, \
             tc.tile_pool(name="st_sb", bufs=4) as st_sb, \
             tc.tile_pool(name="ps", bufs=4, space="PSUM") as ps_pool:
            atq = sb.tile([128, 8, ROWS], FP16, tag="atq")
            nc.sync.dma_start(out=atq, in_=atq_d.rearrange("(hc p) i -> p hc i", p=128))
            rdn = sb.tile([128, 8, ROWS], FP16, tag="rdn")
            nc.sync.dma_start(out=rdn, in_=rdn_d.rearrange("(hc p) i -> p hc i", p=128))
            atn = sb.tile([128, 8, ROWS], FP16, tag="atn")
            nc.vector.tensor_tensor(out=atn, in0=atq, in1=rdn, op=OP.mult)
            wo = sb.tile([128, 8, D], FP16, tag="wo")
            nc.sync.dma_start(out=wo, in_=wo_d.rearrange("(hc p) m -> p hc m", p=128))
            x_sb = sb.tile([128, 4, D], F32, tag="x")
            nc.sync.dma_start(out=x_sb, in_=xr_d.rearrange("(ic p) m -> p ic m", p=128))
            gb = sb.tile([128, D], F32, tag="gb")
            nc.sync.dma_start(
                out=gb,
                in_=bass.AP(tensor=g_d.tensor, offset=g_d.offset, ap=[[0, 128]] + g_d.ap),
            )
            bb = sb.tile([128, D], F32, tag="bb")
            nc.sync.dma_start(
                out=bb,
                in_=bass.AP(tensor=bt_d.tensor, offset=bt_d.offset, ap=[[0, 128]] + bt_d.ap),
            )
            eps_t = sb.tile([128, 1], F32, tag="eps")
            nc.vector.memset(eps_t, LN_EPS)

            for ic in range(4):
                yt = yt_sb.tile([128, D], F32, tag="yt")
                for mh in range(2):
                    o_ps = ps_pool.tile([128, 512], F32, tag="o")
                    for hc in range(8):
                        nc.tensor.matmul(
                            o_ps,
                            atn[:, hc, ic * 128:(ic + 1) * 128],
                            wo[:, hc, mh * 512:(mh + 1) * 512],
                            start=(hc == 0), stop=(hc == 7),
                        )
                    nc.vector.tensor_tensor(
                        out=yt[:, mh * 512:(mh + 1) * 512],
                        in0=o_ps,
                        in1=x_sb[:, ic, mh * 512:(mh + 1) * 512],
                        op=OP.add,
                    )
                stats = st_sb.tile([128, 2, 6], F32, tag="stats")
                for sg in range(2):
                    nc.vector.bn_stats(
                        out=stats[:, sg, :], in_=yt[:, sg * 512:(sg + 1) * 512]
                    )
                mv = st_sb.tile([128, 2], F32, tag="mv")
                nc.vector.bn_aggr(out=mv, in_=stats)
                rstd = st_sb.tile([128, 1], F32, tag="rstd")
                nc.scalar.activation(
                    out=rstd, in_=mv[:, 1:2], func=AF.Sqrt, bias=eps_t, scale=1.0
                )
                nc.vector.reciprocal(out=rstd, in_=rstd)
                nc.vector.tensor_scalar(
                    out=yt, in0=yt, scalar1=mv[:, 0:1], scalar2=rstd,
                    op0=OP.subtract, op1=OP.mult,
                )
                nc.vector.tensor_tensor(out=yt, in0=yt, in1=gb, op=OP.mult)
                nc.vector.tensor_tensor(out=yt, in0=yt, in1=bb, op=OP.add)
                nc.sync.dma_start(out=y_d[ic * 128:(ic + 1) * 128, :], in_=yt)
    nc.compile()
    return nc


def _prep_a(x, w_q, w_k, w_v):
    xt = np.ascontiguousarray(x.transpose(0, 2, 1)).astype(FP16_NP)  # [B, D, L]

    def w_slice(w, c):
        return np.ascontiguousarray(
            w[2 * c:2 * c + 2].transpose(1, 0, 2).reshape(D, 2 * DQ)
        ).astype(FP16_NP)

    return [
        {
            "xt": xt,
            "wq": w_slice(w_q, c),
            "wk": w_slice(w_k, c),
            "wv": w_slice(w_v, c),
        }
        for c in range(N_CORES)
    ]


def _prep_b(res_a_results, x, w_o, ln_gamma, ln_beta):
    at_full = np.concatenate(
        [res_a_results[c]["at"][:, :128, :] for c in range(N_CORES)], axis=1
    )  # [B, H*DQ, L]
    den = np.stack(
        [res_a_results[c]["at"][:, 128:130, :] for c in range(N_CORES)], axis=1
    ).reshape(B, H, L)
    rdn_full = np.repeat((1.0 / den).astype(np.float32), DQ, axis=1)  # [B, H*DQ, L]

    ROWS = B * L // N_CORES
    wo_flat = np.ascontiguousarray(w_o.reshape(H * DQ, D)).astype(FP16_NP)
    # exact power-of-two rescale keeps both factors in fp16 range;
    # it cancels exactly in the on-device product
    at_bf = (at_full * (1.0 / 64.0)).astype(FP16_NP)
    rdn_bf = (rdn_full * 64.0).astype(FP16_NP)
    in_maps_b = []
    for c in range(N_CORES):
        b = c // (N_CORES // B)
        l0 = (c % (N_CORES // B)) * ROWS
        in_maps_b.append(
            {
                "atq": np.ascontiguousarray(at_bf[b][:, l0:l0 + ROWS]),
                "rdn": np.ascontiguousarray(rdn_bf[b][:, l0:l0 + ROWS]),
                "xr": np.ascontiguousarray(x[b, l0:l0 + ROWS]),
                "wo": wo_flat,
                "gamma": ln_gamma,
                "beta": ln_beta,
            }
        )
    return in_maps_b


def kernel(x, w_q, w_k, w_v, w_o, ln_gamma, ln_beta):
    x = np.asarray(x, dtype=np.float32)
    w_q = np.asarray(w_q, dtype=np.float32)
    w_k = np.asarray(w_k, dtype=np.float32)
    w_v = np.asarray(w_v, dtype=np.float32)
    w_o = np.asarray(w_o, dtype=np.float32)
    ln_gamma = np.asarray(ln_gamma, dtype=np.float32)
    ln_beta = np.asarray(ln_beta, dtype=np.float32)

    if "a" not in _cache:
        _cache["a"] = _build_phase_a()
    if "b" not in _cache:
        _cache["b"] = _build_phase_b()

    in_maps_a = _prep_a(x, w_q, w_k, w_v)
    res_a = run_bass_kernel_spmd(
        _cache["a"], in_maps_a, core_ids=list(range(N_CORES)), trace=False
    )
    in_maps_b = _prep_b(res_a.results, x, w_o, ln_gamma, ln_beta)
    res_b = run_bass_kernel_spmd(
        _cache["b"], in_maps_b, core_ids=list(range(N_CORES)), trace=False
    )
    y = np.concatenate([res_b.results[c]["y"] for c in range(N_CORES)], axis=0)
    return y.reshape(B, L, D)



# revision 6
# speedup vs baseline: 1.0593x; 1.0593x over previous
"""Multi-head attention block (QKV proj + softmax attention + out proj +
residual + LayerNorm) on 8 Trainium2 NeuronCores.

Sharding:
  Phase A: head-parallel — core c computes heads (2c, 2c+1) for both batch
           elements: Q/K/V projections, scores (transposed layout), exp,
           unnormalized P@V and softmax denominators.
  Phase B: row-parallel — core c computes 512 rows of the flattened (B*L)
           output: per-head normalization, output projection (contracting
           over all 16 heads), residual add and LayerNorm.

Phase A is paced by the activation engine (128 exp calls over the score
tiles ~= 147us); everything else is scheduled around that stream: the
exp stream starts as early as possible (only the minimal projection
prefix runs up front) and all remaining projection work is emitted as
single-matmul filler units between attention steps, with milestone
forcing so a filler can never be needed before it was emitted.

Matmul operands in fp16 (PE streams 2-byte operands at full rate); all
accumulation in fp32 PSUM; softmax/LayerNorm arithmetic in fp32.
exp uses a fixed -8.0 bias to keep fp16 P values in range; it cancels in
the softmax normalization since denominators use the same biased values.
The two per-head score matmuls share disjoint PE row groups (K=64 at
base partitions 0/64) and execute concurrently.
"""

import os
import sys

if "/opt/trn_rl_repo" not in sys.path:
    sys.path.insert(0, "/opt/trn_rl_repo")

import ml_dtypes
import numpy as np

import concourse.bass as bass
import concourse.tile as tile
from concourse import bacc, mybir
from concourse.bass_utils import run_bass_kernel_spmd
from concourse.masks import make_identity

B, L, D, H, DQ = 2, 2048, 1024, 16, 64
N_CORES = 8
LN_EPS = 1e-5
F32 = mybir.dt.float32
FP16 = mybir.dt.float16
AF = mybir.ActivationFunctionType
OP = mybir.AluOpType
FP16_NP = np.float16

_cache = {}


def _build_phase_a():
    if os.environ.get("PHASEA", "v2") == "base":
        return _build_phase_a_base()
    return _build_phase_a_v2(
        chunk_dma=os.environ.get("A_CHUNK_DMA", "1") == "1",
        sched=os.environ.get("A_SCHED", "1") == "1",
        pool_split=os.environ.get("A_POOLSPLIT", "1") == "1",
        pair_trans=os.environ.get("A_PAIRTRANS", "1") == "1",
        lookahead=os.environ.get("A_LOOKAHEAD", "1") == "1",
    )


def _build_phase_a_v2(chunk_dma=True, sched=True, pool_split=True, pair_trans=True,
                      lookahead=True):
    nc = bacc.Bacc("TRN2", target_bir_lowering=False, debug=False, num_devices=N_CORES)
    xt_d = nc.dram_tensor("xt", [B, D, L], FP16, kind="ExternalInput").ap()
    wq_d = nc.dram_tensor("wq", [D, 128], FP16, kind="ExternalInput").ap()
    wk_d = nc.dram_tensor("wk", [D, 128], FP16, kind="ExternalInput").ap()
    wv_d = nc.dram_tensor("wv", [D, 128], FP16, kind="ExternalInput").ap()
    # rows 0..127: A^T (2 heads x 64), rows 128..129: softmax denominators
    at_d = nc.dram_tensor("at", [B, 130, L], F32, kind="ExternalOutput").ap()

    with tile.TileContext(nc) as tc:
        n_mm = 2 if pool_split else 3
        with tc.tile_pool(name="singles", bufs=1) as singles, \
             tc.tile_pool(name="xt_pool", bufs=2) as xt_pool, \
             tc.tile_pool(name="proj_sb", bufs=2) as proj_sb, \
             tc.tile_pool(name="pt_sb", bufs=6) as pt_sb, \
             tc.tile_pool(name="out_sb", bufs=3) as out_sb, \
             tc.tile_pool(name="ps_mm", bufs=n_mm, space="PSUM") as ps_mm, \
             tc.tile_pool(name="ps_apv", bufs=2, space="PSUM") as ps_apv:
            ps_st_cm = None
            ps_st = None
            if pool_split:
                ps_st_cm = tc.tile_pool(name="ps_st", bufs=2, space="PSUM")
                ps_st = ps_st_cm.__enter__()
            st_pool = ps_st if pool_split else ps_mm
            st_tag = "st" if pool_split else "mm"
            w_sb = {}
            for nm, d in (("wq", wq_d), ("wk", wk_d), ("wv", wv_d)):
                t = singles.tile([128, 8, 128], FP16, tag=nm)
                nc.sync.dma_start(out=t, in_=d.rearrange("(mc p) h -> p mc h", p=128))
                w_sb[nm] = t

            # x^T loaded in [128, 512]-column chunks, it-major then mc, so the
            # earliest projection groups unblock as soon as possible
            xt = {}
            for b in range(B):
                xt[b] = xt_pool.tile([128, 8, L], FP16, tag="xt", name=f"xt{b}")
            xt_r = {b: xt_d[b].rearrange("(mc p) l -> p mc l", p=128) for b in range(B)}
            if chunk_dma:
                for b in range(B):
                    for it in range(4):
                        sl = slice(it * 512, (it + 1) * 512)
                        for mc in range(8):
                            nc.sync.dma_start(
                                out=xt[b][:, mc, sl], in_=xt_r[b][:, mc, sl]
                            )
            else:
                for b in range(B):
                    for mc in range(8):
                        nc.sync.dma_start(out=xt[b][:, mc, :], in_=xt_r[b][:, mc, :])

            ident_f = singles.tile([128, 128], F32, tag="ident_f")
            make_identity(nc, ident_f)
            ident = singles.tile([128, 128], FP16, tag="ident")
            nc.vector.tensor_copy(out=ident, in_=ident_f)
            exp_bias = singles.tile([128, 1], F32, tag="exp_bias")
            nc.vector.memset(exp_bias, -8.0)
            ones_col = singles.tile([128, 2, 16, 1], FP16, tag="ones")
            nc.vector.memset(ones_col, 1.0)

            qt, kt, vt, vsb = {}, {}, {}, {}
            for b in range(B):
                qt[b] = proj_sb.tile([128, L], FP16, tag="qt", name=f"qt{b}")
                kt[b] = proj_sb.tile([128, L], FP16, tag="kt", name=f"kt{b}")
                vt[b] = proj_sb.tile([128, L], FP16, tag="vt", name=f"vt{b}")
                vsb[b] = proj_sb.tile([128, 2, 16, 128], FP16, tag="vsb", name=f"vsb{b}")

            # ---- filler work units (single PE instruction each) ----
            # emitted between attention steps; `due` = latest attention slot
            # (global exp index) before which the unit must be in the queue.
            def proj_units(b, dst, w, it, due):
                sl = slice(it * 512, (it + 1) * 512)
                cell = {}

                def mk(mc):
                    def fn():
                        if mc == 0:
                            cell["ps"] = ps_mm.tile([128, 512], F32, tag="mm", name="ps")
                        nc.tensor.matmul(
                            cell["ps"], w[:, mc, :], xt[b][:, mc, sl],
                            start=(mc == 0), stop=(mc == 7),
                        )
                    return fn

                units = [(due, mk(mc)) for mc in range(8)]

                def cp():
                    nc.vector.tensor_copy(out=dst[:, sl], in_=cell.pop("ps"))

                units.append((due, cp))
                return units

            def v_init_unit(b, due):
                def fn():
                    nc.vector.memset(vsb[b], 0.0)
                    nc.vector.tensor_copy(
                        out=vsb[b][:, :, :, 64:65], in_=ones_col[:, :, :, :]
                    )
                return [(due, fn)]

            def v_transpose_units(b, jc, due):
                if pair_trans:
                    cell = {}

                    def t(h):
                        def fn():
                            if h == 0:
                                cell["ps"] = ps_mm.tile([128, 512], FP16, tag="mm", name="ps")
                            nc.tensor.transpose(
                                cell["ps"][:, h * 64:(h + 1) * 64],
                                vt[b][h * 64:(h + 1) * 64, jc * 128:(jc + 1) * 128],
                                ident[h * 64:(h + 1) * 64, h * 64:(h + 1) * 64],
                            )
                        return fn

                    def cp():
                        ps = cell.pop("ps")
                        nc.vector.tensor_copy(out=vsb[b][:, 0, jc, 0:64], in_=ps[:, 0:64])
                        nc.vector.tensor_copy(out=vsb[b][:, 1, jc, 0:64], in_=ps[:, 64:128])

                    return [(due, t(0)), (due, t(1)), (due, cp)]

                def one(h):
                    def fn():
                        ps = ps_mm.tile([128, 512], FP16, tag="mm", name="ps")
                        nc.tensor.transpose(
                            ps[:, 0:64],
                            vt[b][h * 64:(h + 1) * 64, jc * 128:(jc + 1) * 128],
                            ident[h * 64:(h + 1) * 64, h * 64:(h + 1) * 64],
                        )
                        nc.vector.tensor_copy(out=vsb[b][:, h, jc, 0:64], in_=ps[:, 0:64])
                    return fn

                return [(due, one(0)), (due, one(1))]

            # Build the work list in due order. Slot numbering: global exp
            # index g = 64*b + 16*it + jc.
            work = []
            for b in range(B):
                base = 64 * b
                work += v_init_unit(b, base - 14)
                for it in range(4):
                    # kt block for jc is written by k-proj group jc//4, read
                    # when score(jc) issues (one slot ahead of exp(jc))
                    work += proj_units(b, kt[b], w_sb["wk"], it, base + 4 * it - 2)
                for it in range(4):
                    work += proj_units(b, qt[b], w_sb["wq"], it, base + 16 * it - 2)
                for it in range(4):
                    work += proj_units(b, vt[b], w_sb["wv"], it, base + 4 * it - 1)
                for jc in range(16):
                    work += v_transpose_units(b, jc, base + jc)
            if sched:
                work.sort(key=lambda u: u[0])
            else:
                # no interleaving: emit everything up front (due -inf)
                work = [(-1000, fn) for (_d, fn) in work]
            widx = [0]

            def emit_work(slot, pace):
                n = 0
                while widx[0] < len(work) and (
                    work[widx[0]][0] <= slot or n < pace
                ):
                    work[widx[0]][1]()
                    widx[0] += 1
                    n += 1

            if not sched:
                emit_work(10**9, 10**9)

            def attention():
                slot = 0
                sts = {}

                def score_pair(b, it, jc):
                    # one ST tile holds both heads' scores for this jc — the
                    # two K=64 matmuls sit in disjoint PE row groups (base
                    # partitions 0 and 64) and execute concurrently
                    i_sl = slice(it * 512, (it + 1) * 512)
                    st = st_pool.tile([128, 1024], F32, tag=st_tag, name="st")
                    for h in range(2):
                        hs = slice(h * 64, (h + 1) * 64)
                        nc.tensor.matmul(
                            st[:, h * 512:(h + 1) * 512],
                            kt[b][hs, jc * 128:(jc + 1) * 128],
                            qt[b][hs, i_sl],
                            start=True, stop=True,
                        )
                    return st

                for b in range(B):
                    for it in range(4):
                        i_sl = slice(it * 512, (it + 1) * 512)
                        apv = [
                            ps_apv.tile([128, 512], F32, tag="apv", name=f"apv{_h}")
                            for _h in range(2)
                        ]
                        ptts = {}

                        def pv_pair(jc):
                            ptt = ptts.pop(jc)
                            for h in range(2):
                                nc.tensor.matmul(
                                    apv[h],
                                    vsb[b][:, h, jc, :],
                                    ptt[:, h * 512:(h + 1) * 512],
                                    start=(jc == 0),
                                    stop=(jc == 15),
                                )

                        if lookahead:
                            if it == 0:
                                emit_work(slot - 1, 0)
                                sts[0] = score_pair(b, 0, 0)
                        else:
                            emit_work(slot - 1, 0)
                            sts[0] = score_pair(b, it, 0)
                            sts[1] = score_pair(b, it, 1)
                        for jc in range(16):
                            if lookahead:
                                if jc + 1 < 16:
                                    sts[jc + 1] = score_pair(b, it, jc + 1)
                                elif it + 1 < 4:
                                    sts[-1] = score_pair(b, it + 1, 0)
                            ptts[jc] = pt_sb.tile([128, 1024], FP16, tag="pt", name="pt")
                            nc.scalar.activation(
                                out=ptts[jc], in_=sts.pop(jc), func=AF.Exp,
                                scale=1.0 / (DQ ** 0.5), bias=exp_bias,
                            )
                            if (not lookahead) and jc + 2 < 16:
                                sts[jc + 2] = score_pair(b, it, jc + 2)
                            if jc >= 1:
                                pv_pair(jc - 1)
                            emit_work(slot, 2)
                            slot += 1
                        pv_pair(15)
                        if -1 in sts:
                            sts[0] = sts.pop(-1)
                        for h in range(2):
                            o_sb = out_sb.tile([65, 512], F32, tag="o")
                            nc.vector.tensor_copy(out=o_sb, in_=apv[h][0:65, :])
                            nc.sync.dma_start(
                                out=at_d[b, h * 64:(h + 1) * 64, i_sl], in_=o_sb[0:64, :]
                            )
                            nc.sync.dma_start(
                                out=at_d[b, 128 + h:129 + h, i_sl], in_=o_sb[64:65, :]
                            )

            attention()
            emit_work(10**9, 10**9)
            if ps_st_cm is not None:
                ps_st_cm.__exit__(None, None, None)
    nc.compile()
    return nc


def _build_phase_a_base():
    nc = bacc.Bacc("TRN2", target_bir_lowering=False, debug=False, num_devices=N_CORES)
    xt_d = nc.dram_tensor("xt", [B, D, L], FP16, kind="ExternalInput").ap()
    wq_d = nc.dram_tensor("wq", [D, 128], FP16, kind="ExternalInput").ap()
    wk_d = nc.dram_tensor("wk", [D, 128], FP16, kind="ExternalInput").ap()
    wv_d = nc.dram_tensor("wv", [D, 128], FP16, kind="ExternalInput").ap()
    at_d = nc.dram_tensor("at", [B, 130, L], F32, kind="ExternalOutput").ap()

    with tile.TileContext(nc) as tc:
        with tc.tile_pool(name="singles", bufs=1) as singles, \
             tc.tile_pool(name="xt_pool", bufs=2) as xt_pool, \
             tc.tile_pool(name="proj_sb", bufs=2) as proj_sb, \
             tc.tile_pool(name="pt_sb", bufs=6) as pt_sb, \
             tc.tile_pool(name="out_sb", bufs=3) as out_sb, \
             tc.tile_pool(name="ps_mm", bufs=3, space="PSUM") as ps_mm, \
             tc.tile_pool(name="ps_apv", bufs=2, space="PSUM") as ps_apv:
            w_sb = {}
            for nm, d in (("wq", wq_d), ("wk", wk_d), ("wv", wv_d)):
                t = singles.tile([128, 8, 128], FP16, tag=nm)
                nc.sync.dma_start(out=t, in_=d.rearrange("(mc p) h -> p mc h", p=128))
                w_sb[nm] = t
            ident_f = singles.tile([128, 128], F32, tag="ident_f")
            make_identity(nc, ident_f)
            ident = singles.tile([128, 128], FP16, tag="ident")
            nc.vector.tensor_copy(out=ident, in_=ident_f)
            exp_bias = singles.tile([128, 1], F32, tag="exp_bias")
            nc.vector.memset(exp_bias, -8.0)

            xt, qt, kt, vt, vsb = {}, {}, {}, {}, {}
            for b in range(B):
                xt[b] = xt_pool.tile([128, 8, L], FP16, tag="xt", name=f"xt{b}")
                for mc in range(8):
                    nc.sync.dma_start(
                        out=xt[b][:, mc, :],
                        in_=xt_d[b].rearrange("(mc p) l -> p mc l", p=128)[:, mc, :],
                    )
                qt[b] = proj_sb.tile([128, L], FP16, tag="qt", name=f"qt{b}")
                kt[b] = proj_sb.tile([128, L], FP16, tag="kt", name=f"kt{b}")
                vt[b] = proj_sb.tile([128, L], FP16, tag="vt", name=f"vt{b}")
                vsb[b] = proj_sb.tile([128, 2, 16, 128], FP16, tag="vsb", name=f"vsb{b}")

            def proj_group(b, dst, w, it):
                sl = slice(it * 512, (it + 1) * 512)
                ps = ps_mm.tile([128, 512], F32, tag="mm", name="ps")
                for mc in range(8):
                    nc.tensor.matmul(
                        ps, w[:, mc, :], xt[b][:, mc, sl],
                        start=(mc == 0), stop=(mc == 7),
                    )
                nc.vector.tensor_copy(out=dst[:, sl], in_=ps)

            def v_init(b):
                nc.vector.memset(vsb[b], 0.0)
                nc.vector.tensor_copy(
                    out=vsb[b][:, :, :, 64:65], in_=ones_col[:, :, :, :]
                )

            def v_transpose(b, h, jc):
                ps = ps_mm.tile([128, 512], FP16, tag="mm", name="ps")
                nc.tensor.transpose(
                    ps[:, 0:64],
                    vt[b][h * 64:(h + 1) * 64, jc * 128:(jc + 1) * 128],
                    ident[h * 64:(h + 1) * 64, h * 64:(h + 1) * 64],
                )
                nc.vector.tensor_copy(out=vsb[b][:, h, jc, 0:64], in_=ps[:, 0:64])

            ones_col = singles.tile([128, 2, 16, 1], FP16, tag="ones")
            nc.vector.memset(ones_col, 1.0)

            for it in range(4):
                proj_group(0, kt[0], w_sb["wk"], it)
            proj_group(0, qt[0], w_sb["wq"], 0)
            for it in range(4):
                proj_group(0, vt[0], w_sb["wv"], it)
            v_init(0)
            for jc in range(16):
                for h in range(2):
                    v_transpose(0, h, jc)

            filler = []
            for it in range(1, 4):
                filler.append(lambda it=it: proj_group(0, qt[0], w_sb["wq"], it))
            for it in range(4):
                filler.append(lambda it=it: proj_group(1, qt[1], w_sb["wq"], it))
            for it in range(4):
                filler.append(lambda it=it: proj_group(1, kt[1], w_sb["wk"], it))
            for it in range(4):
                filler.append(lambda it=it: proj_group(1, vt[1], w_sb["wv"], it))
            filler.append(lambda: v_init(1))
            for jc in range(16):
                filler.append(lambda jc=jc: (v_transpose(1, 0, jc), v_transpose(1, 1, jc)))

            nonlocal_pace = [0.0]

            def attention(b, emit_filler):
                for it in range(4):
                    i_sl = slice(it * 512, (it + 1) * 512)
                    apv = [
                        ps_apv.tile([128, 512], F32, tag="apv", name=f"apv{_h}")
                        for _h in range(2)
                    ]
                    def score_pair(jc):
                        st = ps_mm.tile([128, 1024], F32, tag="mm", name="st")
                        for h in range(2):
                            hs = slice(h * 64, (h + 1) * 64)
                            nc.tensor.matmul(
                                st[:, h * 512:(h + 1) * 512],
                                kt[b][hs, jc * 128:(jc + 1) * 128],
                                qt[b][hs, i_sl],
                                start=True, stop=True,
                            )
                        return st

                    def pv_pair(jc, ptt):
                        for h in range(2):
                            nc.tensor.matmul(
                                apv[h],
                                vsb[b][:, h, jc, :],
                                ptt[:, h * 512:(h + 1) * 512],
                                start=(jc == 0),
                                stop=(jc == 15),
                            )

                    sts = {0: score_pair(0)}
                    if 1 < 16:
                        sts[1] = score_pair(1)
                    ptts = {}
                    for jc in range(16):
                        ptts[jc] = pt_sb.tile([128, 1024], FP16, tag="pt", name="pt")
                        nc.scalar.activation(
                            out=ptts[jc], in_=sts.pop(jc), func=AF.Exp,
                            scale=1.0 / (DQ ** 0.5), bias=exp_bias,
                        )
                        if jc + 2 < 16:
                            sts[jc + 2] = score_pair(jc + 2)
                        if jc >= 1:
                            pv_pair(jc - 1, ptts.pop(jc - 1))
                        if emit_filler:
                            nonlocal_pace[0] += 35.0 / 64.0
                            while filler and nonlocal_pace[0] >= 1.0:
                                nonlocal_pace[0] -= 1.0
                                filler.pop(0)()
                    pv_pair(15, ptts.pop(15))
                    for h in range(2):
                        o_sb = out_sb.tile([65, 512], F32, tag="o")
                        nc.vector.tensor_copy(out=o_sb, in_=apv[h][0:65, :])
                        nc.sync.dma_start(
                            out=at_d[b, h * 64:(h + 1) * 64, i_sl], in_=o_sb[0:64, :]
                        )
                        nc.sync.dma_start(
                            out=at_d[b, 128 + h:129 + h, i_sl], in_=o_sb[64:65, :]
                        )

            attention(0, True)
            while filler:
                filler.pop(0)()
            attention(1, False)
    nc.compile()
    return nc


def _build_phase_b():
    nc = bacc.Bacc("TRN2", target_bir_lowering=False, debug=False, num_devices=N_CORES)
    ROWS = B * L // N_CORES  # 512
    # all inputs pre-chunked by ic (block of 128 output rows) host-side so
    # each chunk is one contiguous DMA and compute starts after chunk 0
    atq_d = nc.dram_tensor("atq", [4, H * DQ, 128], FP16, kind="ExternalInput").ap()
    rdn_d = nc.dram_tensor("rdn", [4, H * DQ, 128], FP16, kind="ExternalInput").ap()
    xr_d = nc.dram_tensor("xr", [4, 128, D], FP16, kind="ExternalInput").ap()
    wo_d = nc.dram_tensor("wo", [H * DQ, D], FP16, kind="ExternalInput").ap()
    g_d = nc.dram_tensor("gamma", [D], F32, kind="ExternalInput").ap()
    bt_d = nc.dram_tensor("beta", [D], F32, kind="ExternalInput").ap()
    y_d = nc.dram_tensor("y", [ROWS, D], F32, kind="ExternalOutput").ap()

    with tile.TileContext(nc) as tc:
        with tc.tile_pool(name="sb", bufs=1) as sb, \
             tc.tile_pool(name="in_sb", bufs=3) as in_sb, \
             tc.tile_pool(name="yt_sb", bufs=2) as yt_sb, \
             tc.tile_pool(name="st_sb", bufs=4) as st_sb, \
             tc.tile_pool(name="ps", bufs=4, space="PSUM") as ps_pool:
            # ic0 inputs first so the first matmul chain unblocks early,
            # then wo (needed progressively), then the rest ic-major
            atq, rdn, x_sb = {}, {}, {}

            def load_chunk(ic):
                atq[ic] = in_sb.tile([128, 8, 128], FP16, tag="atq", name=f"atq{ic}")
                nc.sync.dma_start(
                    out=atq[ic],
                    in_=atq_d[ic].rearrange("(hc p) i -> p hc i", p=128),
                )
                rdn[ic] = in_sb.tile([128, 8, 128], FP16, tag="rdn", name=f"rdn{ic}")
                nc.sync.dma_start(
                    out=rdn[ic],
                    in_=rdn_d[ic].rearrange("(hc p) i -> p hc i", p=128),
                )

            load_chunk(0)
            wo = sb.tile([128, 8, D], FP16, tag="wo")
            wo_r = wo_d.rearrange("(hc p) m -> p hc m", p=128)
            for hc2 in range(4):
                nc.sync.dma_start(
                    out=wo[:, 2 * hc2:2 * hc2 + 2, :],
                    in_=wo_r[:, 2 * hc2:2 * hc2 + 2, :],
                )
            x_sb[0] = in_sb.tile([128, D], FP16, tag="x", name="x0")
            nc.sync.dma_start(out=x_sb[0], in_=xr_d[0])
            for ic in range(1, 4):
                load_chunk(ic)
                x_sb[ic] = in_sb.tile([128, D], FP16, tag="x", name=f"x{ic}")
                nc.sync.dma_start(out=x_sb[ic], in_=xr_d[ic])
            gb = sb.tile([128, D], F32, tag="gb")
            nc.sync.dma_start(
                out=gb,
                in_=bass.AP(tensor=g_d.tensor, offset=g_d.offset, ap=[[0, 128]] + g_d.ap),
            )
            bb = sb.tile([128, D], F32, tag="bb")
            nc.sync.dma_start(
                out=bb,
                in_=bass.AP(tensor=bt_d.tensor, offset=bt_d.offset, ap=[[0, 128]] + bt_d.ap),
            )
            eps_t = sb.tile([128, 1], F32, tag="eps")
            nc.vector.memset(eps_t, LN_EPS)

            for ic in range(4):
                atn = in_sb.tile([128, 8, 128], FP16, tag="atn", name=f"atn{ic}")
                nc.vector.tensor_tensor(out=atn, in0=atq[ic], in1=rdn[ic], op=OP.mult)
                yt = yt_sb.tile([128, D], F32, tag="yt")
                for mh in range(2):
                    o_ps = ps_pool.tile([128, 512], F32, tag="o")
                    for hc in range(8):
                        nc.tensor.matmul(
                            o_ps,
                            atn[:, hc, :],
                            wo[:, hc, mh * 512:(mh + 1) * 512],
                            start=(hc == 0), stop=(hc == 7),
                        )
                    nc.vector.tensor_tensor(
                        out=yt[:, mh * 512:(mh + 1) * 512],
                        in0=o_ps,
                        in1=x_sb[ic][:, mh * 512:(mh + 1) * 512],
                        op=OP.add,
                    )
                stats = st_sb.tile([128, 2, 6], F32, tag="stats")
                for sg in range(2):
                    nc.vector.bn_stats(
                        out=stats[:, sg, :], in_=yt[:, sg * 512:(sg + 1) * 512]
                    )
                mv = st_sb.tile([128, 2], F32, tag="mv")
                nc.vector.bn_aggr(out=mv, in_=stats)
                rstd = st_sb.tile([128, 1], F32, tag="rstd")
                nc.scalar.activation(
                    out=rstd, in_=mv[:, 1:2], func=AF.Sqrt, bias=eps_t, scale=1.0
                )
                nc.vector.reciprocal(out=rstd, in_=rstd)
                nc.vector.tensor_scalar(
                    out=yt, in0=yt, scalar1=mv[:, 0:1], scalar2=rstd,
                    op0=OP.subtract, op1=OP.mult,
                )
                nc.vector.tensor_tensor(out=yt, in0=yt, in1=gb, op=OP.mult)
                nc.vector.tensor_tensor(out=yt, in0=yt, in1=bb, op=OP.add)
                nc.sync.dma_start(out=y_d[ic * 128:(ic + 1) * 128, :], in_=yt)
    nc.compile()
    return nc


def _prep_a(x, w_q, w_k, w_v):
    xt = np.ascontiguousarray(x.transpose(0, 2, 1)).astype(FP16_NP)  # [B, D, L]

    def w_slice(w, c):
        return np.ascontiguousarray(
            w[2 * c:2 * c + 2].transpose(1, 0, 2).reshape(D, 2 * DQ)
        ).astype(FP16_NP)

    return [
        {
            "xt": xt,
            "wq": w_slice(w_q, c),
            "wk": w_slice(w_k, c),
            "wv": w_slice(w_v, c),
        }
        for c in range(N_CORES)
    ]


def _prep_b(res_a_results, x, w_o, ln_gamma, ln_beta):
    at_full = np.concatenate(
        [res_a_results[c]["at"][:, :128, :] for c in range(N_CORES)], axis=1
    )  # [B, H*DQ, L]
    den = np.stack(
        [res_a_results[c]["at"][:, 128:130, :] for c in range(N_CORES)], axis=1
    ).reshape(B, H, L)
    rdn_full = np.repeat((1.0 / den).astype(np.float32), DQ, axis=1)  # [B, H*DQ, L]

    ROWS = B * L // N_CORES
    wo_flat = np.ascontiguousarray(w_o.reshape(H * DQ, D)).astype(FP16_NP)
    # exact power-of-two rescale keeps both factors in fp16 range;
    # it cancels exactly in the on-device product
    at_bf = (at_full * (1.0 / 64.0)).astype(FP16_NP)
    rdn_bf = (rdn_full * 64.0).astype(FP16_NP)

    def ic_chunk(a):  # [H*DQ, ROWS] -> [4, H*DQ, 128]
        return np.ascontiguousarray(
            a.reshape(H * DQ, 4, 128).transpose(1, 0, 2)
        )

    in_maps_b = []
    for c in range(N_CORES):
        b = c // (N_CORES // B)
        l0 = (c % (N_CORES // B)) * ROWS
        in_maps_b.append(
            {
                "atq": ic_chunk(at_bf[b][:, l0:l0 + ROWS]),
                "rdn": ic_chunk(rdn_bf[b][:, l0:l0 + ROWS]),
                "xr": np.ascontiguousarray(
                    x[b, l0:l0 + ROWS].reshape(4, 128, D)
                ).astype(FP16_NP),
                "wo": wo_flat,
                "gamma": ln_gamma,
                "beta": ln_beta,
            }
        )
    return in_maps_b


def kernel(x, w_q, w_k, w_v, w_o, ln_gamma, ln_beta):
    x = np.asarray(x, dtype=np.float32)
    w_q = np.asarray(w_q, dtype=np.float32)
    w_k = np.asarray(w_k, dtype=np.float32)
    w_v = np.asarray(w_v, dtype=np.float32)
    w_o = np.asarray(w_o, dtype=np.float32)
    ln_gamma = np.asarray(ln_gamma, dtype=np.float32)
    ln_beta = np.asarray(ln_beta, dtype=np.float32)

    if "a" not in _cache:
        _cache["a"] = _build_phase_a()
    if "b" not in _cache:
        _cache["b"] = _build_phase_b()

    in_maps_a = _prep_a(x, w_q, w_k, w_v)
    res_a = run_bass_kernel_spmd(
        _cache["a"], in_maps_a, core_ids=list(range(N_CORES)), trace=False
    )
    in_maps_b = _prep_b(res_a.results, x, w_o, ln_gamma, ln_beta)
    res_b = run_bass_kernel_spmd(
        _cache["b"], in_maps_b, core_ids=list(range(N_CORES)), trace=False
    )
    y = np.concatenate([res_b.results[c]["y"] for c in range(N_CORES)], axis=0)
    return y.reshape(B, L, D)
